# revision 45
# baseline (speedup 1.0000x reference)
"""Trainium2 Bass kernel for nn_FR_PDP_block (dense_cnn).

Strategy: pure data parallelism, B=16 sharded as 2 samples per core over 8
NeuronCores. All parameters replicated. Device computes in fp16 (fp32 PSUM
accumulation); the final residual add (+x) happens on host in fp32.

Per-core pipeline (channels-on-partitions, C=256 = 2 chunks of 128):
  PW1 (PE matmul) -> padded out1 (zero borders feed conv-tap windows)
  depthwise branches = per-channel MAC taps over the padded layout:
     xy (5x5=25 taps): mostly PE diagonal-matmul accumulation in PSUM,
         remainder on DVE as tensor_scalar product + tensor_tensor add pairs
     xz/yz (5 taps): chunk0 on GPSIMD (scalar_tensor_tensor), chunk1 on DVE
     BN scale folded into tap weights (host); BN shift via drain bias / init
  gate = sigmoid(relu(acc_xz)+relu(acc_yz)) [alpha/beta folded into weights]
  top  = relu(acc_xy) * gate   (relu fused into the PE-psum drain for chunk0)
  PW2 (PE, BN2 scale folded) -> relu drain with accum_out (SE squeeze)
  SE FCs on PE (fp32), sigmoid -> per-(chunk,sample) scale of out2
  store fp16; host adds x in fp32.
"""
import sys
from contextlib import ExitStack

import numpy as np

sys.path.insert(0, "/opt/trn_rl_repo")

import concourse.bacc as bacc
import concourse.mybir as mybir
import concourse.tile as tile
from concourse import bass2jax

EPS = 1e-5
B, C, H, W = 16, 256, 56, 56
HW = H * W          # 3136
BL = 2              # samples per core
NC_ = 8             # cores
PF = 128            # partitions
NK = C // PF        # 2 channel chunks
NT = 448            # one psum bank (8 rows of 56)
NTILES = HW // NT   # 7 per sample
HP, WP = H + 4, W + 4

F16 = mybir.dt.float16
F32 = mybir.dt.float32
A = mybir.AluOpType
AF = mybir.ActivationFunctionType

# taps (dy, dx) offsets in 0..4 (padded coords); center first
XY_TAPS = sorted(
    [(dy, dx) for dy in range(5) for dx in range(5)],
    key=lambda t: (abs(t[0] - 2) + abs(t[1] - 2), t),
)
# xy is split by pixel region: PE computes ntiles [0, XY_PE_NTILES), DVE the
# rest (all 25 taps each, disjoint output regions - no merge needed)
XY_PE_NTILES = {0: 14, 1: 14}
XY_DVE_NTILES = {0: 0, 1: 0}  # DVE region size (rest of the 14 ntiles)
# engine for the 1D branches' accumulation adds, per chunk:
#   "dve"  - products and adds on DVE
#   "pool" - products on DVE, adds on GPSIMD tensor_tensor
XZ_ENGINE = {0: "pool", 1: "dve"}
YZ_ENGINE = {0: "pool", 1: "dve"}


def build_module(n_iters: int = 1, unroll: bool = False):
    nc = bacc.Bacc(None, target_bir_lowering=False)

    with tile.TileContext(nc) as tc, ExitStack() as es:
        # ---------------- DRAM I/O ----------------
        x16 = nc.dram_tensor("x16", [NK, PF, BL, HW], F16, kind="ExternalInput").ap()
        w1 = nc.dram_tensor("w1", [NK, NK, PF, PF], F16, kind="ExternalInput").ap()
        w2 = nc.dram_tensor("w2", [NK, NK, PF, PF], F16, kind="ExternalInput").ap()
        dxy = nc.dram_tensor("dxy", [NK, 25, PF, PF], F16, kind="ExternalInput").ap()
        dyz = nc.dram_tensor("dyz", [5, PF, PF], F16, kind="ExternalInput").ap()
        kxy = nc.dram_tensor("kxy", [NK, PF, 25], F32, kind="ExternalInput").ap()
        kxz = nc.dram_tensor("kxz", [NK, PF, 5], F32, kind="ExternalInput").ap()
        kyz = nc.dram_tensor("kyz", [NK, PF, 5], F32, kind="ExternalInput").ap()
        tv = nc.dram_tensor("tv", [NK, PF, 4], F32, kind="ExternalInput").ap()
        fc1t = nc.dram_tensor("fc1t", [NK, PF, 16], F32, kind="ExternalInput").ap()
        fc1b = nc.dram_tensor("fc1b", [16, 1], F32, kind="ExternalInput").ap()
        fc2t = nc.dram_tensor("fc2t", [NK, 16, PF], F32, kind="ExternalInput").ap()
        fc2b = nc.dram_tensor("fc2b", [NK, PF, 1], F32, kind="ExternalInput").ap()
        y16 = nc.dram_tensor("y16", [NK, PF, BL, HW], F16, kind="ExternalOutput").ap()

        # ---------------- persistent SBUF ----------------
        const = es.enter_context(tc.tile_pool(name="const", bufs=1))
        xsb = [const.tile([PF, BL, HW], F16, tag=f"xsb{k}", name=f"xsb{k}") for k in range(NK)]
        o1p = [const.tile([PF, BL, HP, WP], F16, tag=f"o1p{k}", name=f"o1p{k}") for k in range(NK)]
        accxz = [const.tile([PF, BL, H, W], F16, tag=f"accxz{k}", name=f"accxz{k}") for k in range(NK)]
        accyz = [const.tile([PF, BL, H, W], F16, tag=f"accyz{k}", name=f"accyz{k}") for k in range(NK)]
        accxy = [const.tile([PF, BL * HW], F16, tag=f"accxy{k}", name=f"accxy{k}") for k in range(NK)]
        out2 = [const.tile([PF, BL, HW], F16, tag=f"out2{k}", name=f"out2{k}") for k in range(NK)]
        w1sb = const.tile([PF, NK, NK, PF], F16, tag="w1sb")
        w2sb = const.tile([PF, NK, NK, PF], F16, tag="w2sb")
        dxysb = const.tile([PF, NK, 25, PF], F16, tag="dxysb")
        dyzsb = const.tile([PF, 5, PF], F16, tag="dyzsb")
        kxysb = const.tile([PF, NK, 25], F32, tag="kxysb")
        kxzsb = const.tile([PF, NK, 5], F32, tag="kxzsb")
        kyzsb = const.tile([PF, NK, 5], F32, tag="kyzsb")
        tvsb = const.tile([PF, NK, 4], F32, tag="tvsb")
        fc1tsb = const.tile([PF, NK, 16], F32, tag="fc1tsb")
        fc1bsb = const.tile([16, 1], F32, tag="fc1bsb")
        fc2tsb = const.tile([16, NK, PF], F32, tag="fc2tsb")
        fc2bsb = const.tile([PF, NK, 1], F32, tag="fc2bsb")
        sq = [const.tile([PF, 8], F32, tag=f"sq{k}", name=f"sq{k}") for k in range(NK)]
        sqv = [const.tile([PF, BL], F32, tag=f"sqv{k}", name=f"sqv{k}") for k in range(NK)]
        s1sb = const.tile([16, BL], F32, tag="s1sb")
        sesb = [const.tile([PF, BL], F32, tag=f"sesb{k}", name=f"sesb{k}") for k in range(NK)]

        tmppool = es.enter_context(tc.tile_pool(name="tmppool", bufs=4))

        # ---------------- loads + border zeroing (once) ----------------
        nc.sync.dma_start(out=w1sb[:], in_=w1.rearrange("a b p m -> p a b m"))
        for _b in range(BL):
            for _k in range(NK):
                nc.sync.dma_start(out=xsb[_k][:, _b, :], in_=x16[_k][:, _b, :])
        nc.sync.dma_start(out=w2sb[:], in_=w2.rearrange("a b p m -> p a b m"))
        nc.sync.dma_start(out=kxysb[:], in_=kxy.rearrange("a p t -> p a t"))
        nc.sync.dma_start(out=kxzsb[:], in_=kxz.rearrange("a p t -> p a t"))
        nc.sync.dma_start(out=kyzsb[:], in_=kyz.rearrange("a p t -> p a t"))
        nc.sync.dma_start(out=tvsb[:], in_=tv.rearrange("a p t -> p a t"))
        nc.sync.dma_start(out=fc1tsb[:], in_=fc1t.rearrange("a p t -> p a t"))
        nc.sync.dma_start(out=fc1bsb[:], in_=fc1b)
        nc.sync.dma_start(out=fc2tsb[:], in_=fc2t.rearrange("a p m -> p a m"))
        nc.sync.dma_start(out=fc2bsb[:], in_=fc2b.rearrange("a p o -> p a o"))
        nc.sync.dma_start(out=dxysb[:], in_=dxy.rearrange("a t p m -> p a t m"))
        nc.sync.dma_start(out=dyzsb[:], in_=dyz.rearrange("t p m -> p t m"))
        for k in range(NK):
            for b in range(BL):
                nc.vector.memset(o1p[k][:, b, 0:2, :], 0.0)
                nc.vector.memset(o1p[k][:, b, H + 2:HP, :], 0.0)
                nc.vector.memset(o1p[k][:, b, 2:H + 2, 0:2], 0.0)
                nc.vector.memset(o1p[k][:, b, 2:H + 2, W + 2:WP], 0.0)

        # one shared psum pool: 4 x [128,1024] tiles = all 8 banks
        pwps = es.enter_context(tc.tile_pool(name="pwps", bufs=4, space="PSUM"))
        xyps = pwps

        def dve_branch_taps(acc, k, taps, ksb, kidx, t_ap, add_engine="dve"):
            """Accumulate `taps` into acc[k] (per sample; 3D APs).
            Products on DVE tensor_scalar (4x); adds on DVE or GPSIMD
            tensor_tensor. First tap initializes acc (+t shift if given)."""
            for b in range(BL):
                accf = acc[k][:, b].rearrange("p h w -> p (h w)")
                for i, (dy, dx) in enumerate(taps):
                    win = o1p[k][:, b, dy:dy + H, dx:dx + W]
                    sc = ksb[:, k, kidx[i]:kidx[i] + 1]
                    if i == 0:
                        nc.vector.tensor_scalar(
                            out=accf, in0=win, scalar1=sc,
                            scalar2=t_ap, op0=A.mult,
                            op1=A.add if t_ap is not None else A.bypass)
                    else:
                        tmp = tmppool.tile([PF, HW], F16, tag="dvetmp", name=f"tmp{k}{b}{i}")
                        nc.vector.tensor_scalar(
                            out=tmp[:], in0=win, scalar1=sc, scalar2=None, op0=A.mult)
                        if add_engine == "dma":
                            nc.gpsimd.dma_start(out=accf, in_=tmp[:], accum_op=A.add)
                        elif add_engine == "pool":
                            nc.gpsimd.tensor_add(out=accf, in0=accf, in1=tmp[:])
                        else:
                            nc.vector.tensor_add(out=accf, in0=accf, in1=tmp[:])

        def body(_it=0, first=False):
            if not first:
                for b in range(BL):
                    for k in range(NK):
                        nc.sync.dma_start(out=xsb[k][:, b, :], in_=x16[k][:, b, :])

            # ---- PW1 -> padded out1 ----
            # pair-groups of 2 psum tiles; ki-inner ordering amortizes LDWEIGHTS
            for ko in range(NK):
                pairs = [(b, j0) for b in range(BL) for j0 in range(0, NTILES, 2)]
                # 4 psum tiles per group: 8 matmuls per LDWEIGHTS
                for pg0 in range(0, len(pairs), 4):
                    grp = pairs[pg0:pg0 + 4]
                    tiles = [pwps.tile([PF, 1024], F32, tag="pw", name=f"pw1t{ko}_{pg0}_{gi}")
                             for gi in range(len(grp))]
                    for ki in range(NK):
                        for gi, (b, j0) in enumerate(grp):
                            js = [j0] if j0 + 1 >= NTILES else [j0, j0 + 1]
                            for si, j in enumerate(js):
                                nt = b * NTILES + j
                                nc.tensor.matmul(
                                    tiles[gi][:, si * 512:si * 512 + NT],
                                    lhsT=w1sb[:, ki, ko, :],
                                    rhs=xsb[ki].rearrange("p b n -> p (b n)")[:, nt * NT:(nt + 1) * NT],
                                    start=(ki == 0), stop=(ki == NK - 1),
                                )
                    for gi, (b, j0) in enumerate(grp):
                        js = [j0] if j0 + 1 >= NTILES else [j0, j0 + 1]
                        for si, j in enumerate(js):
                            r0 = j * 8
                            nc.scalar.copy(
                                out=o1p[ko][:, b, 2 + r0:2 + r0 + 8, 2:2 + W],
                                in_=tiles[gi][:, si * 512:si * 512 + NT].rearrange("p (r w) -> p r w", w=W),
                            )

            # ---- depthwise branches ----
            xz_taps = [(2, dx) for dx in (2, 0, 1, 3, 4)]
            yz_taps = [(dy, 2) for dy in (2, 0, 1, 3, 4)]
            xz_idx = [t[1] for t in xz_taps]
            yz_idx = [t[0] for t in yz_taps]

            def xy_pe(k):
                n_nt = XY_PE_NTILES[k]
                for g0 in range(0, n_nt, 8):
                    pairs = [nt0 for nt0 in (g0, g0 + 2, g0 + 4, g0 + 6) if nt0 < n_nt]
                    tiles = {nt0: xyps.tile([PF, 1024], F32, tag="pw",
                                            name=f"xyps{k}_{nt0}") for nt0 in pairs}
                    for ti, (dy, dx) in enumerate(XY_TAPS):
                        for nt0 in pairs:
                            for si in range(2):
                                nt = nt0 + si
                                b, r0 = nt // NTILES, (nt % NTILES) * 8
                                nc.tensor.matmul(
                                    tiles[nt0][:, si * 512:si * 512 + NT],
                                    lhsT=dxysb[:, k, ti, :],
                                    rhs=o1p[k][:, b, r0 + dy:r0 + dy + 8, dx:dx + W],
                                    start=(ti == 0), stop=(ti == 24),
                                )
                    for nt0 in pairs:
                        nc.scalar.activation(
                            out=accxy[k][:, nt0 * NT:(nt0 + 2) * NT].rearrange(
                                "p (s q) -> p s q", q=NT),
                            in_=tiles[nt0][:].rearrange("p (s q) -> p s q", q=512)[:, :, 0:NT],
                            func=AF.Relu, bias=tvsb[:, k, 0:1], scale=1.0,
                        )

            def xy_dve(k):
                """DVE computes ntiles [XY_PE_NTILES[k], +XY_DVE_NTILES[k]):
                all 25 taps, f16 acc, then relu; writes accxy region."""
                n0 = XY_PE_NTILES[k]
                n1 = min(n0 + XY_DVE_NTILES[k], BL * NTILES)
                if n0 >= BL * NTILES:
                    return
                for b in range(BL):
                    jlo = max(0, n0 - b * NTILES)
                    jhi = min(NTILES, n1 - b * NTILES)
                    if jlo >= NTILES or jhi <= jlo:
                        continue
                    r0, r1 = jlo * 8, jhi * 8
                    nrow = r1 - r0
                    accf = accxy[k][:, (b * NTILES + jlo) * NT:(b * NTILES + jhi) * NT]
                    for i, (dy, dx) in enumerate(XY_TAPS):
                        win = o1p[k][:, b, r0 + dy:r0 + nrow + dy, dx:dx + W]
                        sc = kxysb[:, k, i:i + 1]
                        if i == 0:
                            nc.vector.tensor_scalar(
                                out=accf, in0=win, scalar1=sc,
                                scalar2=tvsb[:, k, 0:1], op0=A.mult, op1=A.add)
                        else:
                            tmp = tmppool.tile([PF, nrow * W], F16, tag="dvetmp",
                                               name=f"xytmp{k}{b}{i}")
                            nc.vector.tensor_scalar(
                                out=tmp[:], in0=win, scalar1=sc, scalar2=None, op0=A.mult)
                            nc.vector.tensor_add(out=accf, in0=accf, in1=tmp[:])
                    nc.vector.tensor_scalar(
                        out=accf, in0=accf, scalar1=0.0, scalar2=None, op0=A.max)

            def gate_pre(k, yz_prerelu=False):
                axz = accxz[k].rearrange("p b h w -> p (b h w)")
                ayz = accyz[k].rearrange("p b h w -> p (b h w)")
                nc.vector.tensor_scalar(out=axz, in0=axz, scalar1=0.0, scalar2=None, op0=A.max)
                if not yz_prerelu:
                    nc.vector.tensor_scalar(out=ayz, in0=ayz, scalar1=0.0, scalar2=None, op0=A.max)
                nc.vector.tensor_add(out=axz, in0=axz, in1=ayz)
                nc.scalar.activation(out=axz, in_=axz, func=AF.Sigmoid)

            def gate_mult(k):
                axz = accxz[k].rearrange("p b h w -> p (b h w)")
                nc.vector.tensor_mul(out=accxy[k][:], in0=accxy[k][:], in1=axz)

            def gating(k, yz_prerelu=False):
                gate_pre(k, yz_prerelu)
                gate_mult(k)

            def pe_branch_yz0(n_nt=BL * NTILES):
                accf = accyz[0].rearrange("p b h w -> p (b h w)")
                for g0 in range(0, n_nt, 8):
                    pairs = [nt0 for nt0 in (g0, g0 + 2, g0 + 4, g0 + 6) if nt0 < n_nt]
                    tiles = {nt0: xyps.tile([PF, 1024], F32, tag="pw",
                                            name=f"yzps{nt0}") for nt0 in pairs}
                    for ti, dy in enumerate(yz_idx):
                        for nt0 in pairs:
                            for si in range(2):
                                nt = nt0 + si
                                b, r0 = nt // NTILES, (nt % NTILES) * 8
                                nc.tensor.matmul(
                                    tiles[nt0][:, si * 512:si * 512 + NT],
                                    lhsT=dyzsb[:, ti, :],
                                    rhs=o1p[0][:, b, r0 + dy:r0 + dy + 8, 2:2 + W],
                                    start=(ti == 0), stop=(ti == 4),
                                )
                    for nt0 in pairs:
                        nc.scalar.activation(
                            out=accf[:, nt0 * NT:(nt0 + 2) * NT].rearrange(
                                "p (s q) -> p s q", q=NT),
                            in_=tiles[nt0][:].rearrange("p (s q) -> p s q", q=512)[:, :, 0:NT],
                            func=AF.Relu, bias=tvsb[:, 0, 2:3], scale=1.0,
                        )

            # pool-assisted chunk-0 xz branch first (feeds GPSIMD early)
            dve_branch_taps(accxz, 0, xz_taps, kxzsb, xz_idx, tvsb[:, 0, 1:2],
                            add_engine=XZ_ENGINE[0])
            # chunk-0 yz branch on the PE (diag taps, relu fused in drains)
            pe_branch_yz0()
            # chunk1 xy on PE + DVE region
            xy_pe(1)
            xy_dve(1)
            # chunk1 1D branches on DVE
            dve_branch_taps(accxz, 1, xz_taps, kxzsb, xz_idx, tvsb[:, 1, 1:2],
                            add_engine=XZ_ENGINE[1])
            dve_branch_taps(accyz, 1, yz_taps, kyzsb, yz_idx, tvsb[:, 1, 2:3],
                            add_engine=YZ_ENGINE[1])
            gating(1)
            # chunk0 xy
            xy_pe(0)
            xy_dve(0)
            gating(0, yz_prerelu=True)

            # ---- PW2 + BN2+relu + SE squeeze ----
            # sample-outer (b=1 first) so the first sample's SE/scale/store
            # overlaps the remaining PW2 work
            for b, ko in [(1, 0), (1, 1), (0, 0), (0, 1)]:
                pairs2 = [(b, j0) for j0 in range(0, NTILES, 2)]
                for pg0 in range(0, len(pairs2), 4):
                    grp = pairs2[pg0:pg0 + 4]
                    tiles = [pwps.tile([PF, 1024], F32, tag="pw", name=f"pw2t{ko}_{pg0}_{gi}")
                             for gi in range(len(grp))]
                    for ki in (1, 0):
                        for gi, (b, j0) in enumerate(grp):
                            js = [j0] if j0 + 1 >= NTILES else [j0, j0 + 1]
                            for si, j in enumerate(js):
                                nt = b * NTILES + j
                                nc.tensor.matmul(
                                    tiles[gi][:, si * 512:si * 512 + NT],
                                    lhsT=w2sb[:, ki, ko, :],
                                    rhs=accxy[ki][:, nt * NT:(nt + 1) * NT],
                                    start=(ki == 1), stop=(ki == 0),
                                )
                    for gi, (b, j0) in enumerate(grp):
                        js = [j0] if j0 + 1 >= NTILES else [j0, j0 + 1]
                        ps = tiles[gi]
                        if len(js) == 2:
                            out_ap = out2[ko][:, b, j0 * NT:(j0 + 2) * NT].rearrange(
                                "p (s q) -> p s q", q=NT)
                            in_ap = ps[:].rearrange("p (s q) -> p s q", q=512)[:, :, 0:NT]
                        else:
                            out_ap = out2[ko][:, b, j0 * NT:(j0 + 1) * NT]
                            in_ap = ps[:, 0:NT]
                        nc.scalar.activation(
                            out=out_ap, in_=in_ap,
                            func=AF.Relu, bias=tvsb[:, ko, 3:4], scale=1.0,
                            accum_out=sq[ko][:, b * 4 + j0 // 2:b * 4 + j0 // 2 + 1],
                        )

            # ---- SE + final scale + store, per sample ----
            for b in (1, 0):
                for k in range(NK):
                    nc.vector.tensor_reduce(
                        out=sqv[k][:, b:b + 1], in_=sq[k][:, b * 4:b * 4 + 4],
                        axis=mybir.AxisListType.X, op=A.add)
                ps1 = pwps.tile([16, 2], F32, tag="pw", name=f"ps1_{b}")
                for k in range(NK):
                    nc.tensor.matmul(ps1[:, 0:1], lhsT=fc1tsb[:, k, :], rhs=sqv[k][:, b:b + 1],
                                     start=(k == 0), stop=(k == NK - 1))
                nc.scalar.activation(out=s1sb[:, b:b + 1], in_=ps1[:, 0:1], func=AF.Relu,
                                     bias=fc1bsb[:], scale=1.0)
                for k in range(NK):
                    ps2 = pwps.tile([PF, 2], F32, tag="pw", name=f"ps2_{b}_{k}")
                    nc.tensor.matmul(ps2[:, 0:1], lhsT=fc2tsb[:, k, :], rhs=s1sb[:, b:b + 1])
                    nc.scalar.activation(out=sesb[k][:, b:b + 1], in_=ps2[:, 0:1],
                                         func=AF.Sigmoid, bias=fc2bsb[:, k, :], scale=1.0)
                for k in range(NK):
                    if k == 0:
                        nc.scalar.mul(out2[k][:, b, :], out2[k][:, b, :],
                                      sesb[k][:, b:b + 1])
                    else:
                        nc.vector.tensor_scalar(
                            out=out2[k][:, b, :], in0=out2[k][:, b, :],
                            scalar1=sesb[k][:, b:b + 1], scalar2=None, op0=A.mult)
                    nc.sync.dma_start(out=y16[k][:, b, :], in_=out2[k][:, b, :])

        if n_iters == 1:
            body(first=True)
        elif unroll:
            for i in range(n_iters):
                body(i, first=(i == 0))
        else:
            with tc.For_i(0, n_iters, 1,
                          hint_engines=(mybir.EngineType.PE,
                                        mybir.EngineType.DVE,
                                        mybir.EngineType.Activation)) as it:
                body(it)

    nc.compile()
    return nc


# ---------------------------------------------------------------------------
# host-side preparation
# ---------------------------------------------------------------------------

def _prep(inputs):
    f32 = np.float32
    g = {k: np.asarray(v) for k, v in inputs.items()}

    def fold(p):
        s = (g[f"bn{p}_g"] / np.sqrt(g[f"bn{p}_v"] + EPS)).astype(f32)
        t = (g[f"bn{p}_b"] - g[f"bn{p}_m"] * s).astype(f32)
        return s, t

    s_xy, t_xy = fold("xy")
    s_xz, t_xz = fold("xz")
    s_yz, t_yz = fold("yz")
    s_2, t_2 = fold("2")
    alpha = g["alpha"][0, :, 0, 0].astype(f32)
    beta = g["beta"][0, :, 0, 0].astype(f32)

    kxy = g["xy5_w"][:, 0].astype(f32).copy()
    kxy[:, 1:4, 1:4] += g["xy3_w"][:, 0]
    kxy *= s_xy[:, None, None]
    kxz = g["xz5_w"][:, 0, 0].astype(f32).copy()
    kxz[:, 1:4] += g["xz3_w"][:, 0, 0]
    kxz *= (alpha * s_xz)[:, None]
    kyz = g["yz5_w"][:, 0, :, 0].astype(f32).copy()
    kyz[:, 1:4] += g["yz3_w"][:, 0, :, 0]
    kyz *= (beta * s_yz)[:, None]

    w1t = g["pw1_w"][:, :, 0, 0].T.astype(np.float16)     # [c_in, c_out]
    w2t = (g["pw2_w"][:, :, 0, 0] * s_2[:, None]).T.astype(np.float16)
    w1b = w1t.reshape(NK, PF, NK, PF).transpose(0, 2, 1, 3).copy()  # [ki, ko, 128, 128]
    w2b = w2t.reshape(NK, PF, NK, PF).transpose(0, 2, 1, 3).copy()

    # diag stacks for PE xy taps, in XY_TAPS order
    kxy_o = np.stack([kxy[:, dy, dx] for (dy, dx) in XY_TAPS], axis=1)  # [C,25]
    dxy = np.zeros((NK, 25, PF, PF), np.float16)
    for k in range(NK):
        for t in range(25):
            np.fill_diagonal(dxy[k, t], kxy_o[k * PF:(k + 1) * PF, t].astype(np.float16))
    # diag stack for the chunk-0 yz branch on PE, tap order (2,0,1,3,4)
    dyz = np.zeros((5, PF, PF), np.float16)
    for t, dyi in enumerate((2, 0, 1, 3, 4)):
        np.fill_diagonal(dyz[t], kyz[:PF, dyi].astype(np.float16))

    tv = np.stack([t_xy, alpha * t_xz, beta * t_yz, t_2], axis=1)  # [C,4]

    arrs = {
        "w1": w1b, "w2": w2b, "dxy": dxy, "dyz": dyz,
        "kxy": kxy_o.reshape(NK, PF, 25).astype(f32),
        "kxz": kxz.reshape(NK, PF, 5).astype(f32),
        "kyz": kyz.reshape(NK, PF, 5).astype(f32),
        "tv": tv.reshape(NK, PF, 4).astype(f32),
        "fc1t": (g["fc1_w"].T / HW).astype(f32).reshape(NK, PF, 16),
        "fc1b": g["fc1_b"].astype(f32).reshape(16, 1),
        "fc2t": g["fc2_w"].T.reshape(16, NK, PF).transpose(1, 0, 2).astype(f32).copy(),
        "fc2b": g["fc2_b"].astype(f32).reshape(NK, PF, 1),
    }
    return arrs


_CACHE = {}


def _get_runner():
    if "runner" in _CACHE:
        return _CACHE["runner"]
    import jax
    import jax.core as jcore
    from jax.sharding import Mesh, PartitionSpec, NamedSharding
    from jax.experimental.shard_map import shard_map

    nc = build_module(n_iters=1)
    bass2jax.install_neuronx_cc_hook()

    in_names, out_names, out_avals, out_shapes = [], [], [], []
    for alloc in nc.m.functions[0].allocations:
        if not isinstance(alloc, mybir.MemoryLocationSet):
            continue
        name = alloc.memorylocations[0].name
        if alloc.kind == "ExternalInput":
            if nc.partition_id_tensor is None or name != nc.partition_id_tensor.name:
                in_names.append(name)
        elif alloc.kind == "ExternalOutput":
            out_names.append(name)
            shape = tuple(alloc.tensor_shape)
            dtype = mybir.dt.np(alloc.dtype)
            out_avals.append(jcore.ShapedArray(shape, dtype))
            out_shapes.append((shape, dtype))
    all_in = list(in_names) + list(out_names)
    if nc.partition_id_tensor is not None:
        all_in.append(nc.partition_id_tensor.name)

    def _body(*args):
        operands = list(args)
        if nc.partition_id_tensor is not None:
            operands.append(bass2jax.partition_id_tensor())
        outs = bass2jax._bass_exec_p.bind(
            *operands, out_avals=tuple(out_avals), in_names=tuple(all_in),
            out_names=tuple(out_names), lowering_input_output_aliases=(),
            sim_require_finite=False, sim_require_nnan=False, nc=nc)
        return tuple(outs)

    devices = jax.devices()[:NC_]
    mesh = Mesh(np.asarray(devices), ("core",))
    nspec = len(in_names) + len(out_names)
    fn = jax.jit(
        shard_map(_body, mesh=mesh,
                  in_specs=(PartitionSpec("core"),) * nspec,
                  out_specs=(PartitionSpec("core"),) * len(out_names),
                  check_rep=False),
        keep_unused=True,
    )
    sharding = NamedSharding(mesh, PartitionSpec("core"))
    _CACHE["runner"] = (fn, in_names, out_names, out_shapes, sharding)
    return _CACHE["runner"]


def kernel(**inputs) -> np.ndarray:
    import jax

    fn, in_names, out_names, out_shapes, sharding = _get_runner()
    x = np.asarray(inputs["x"], np.float32)
    arrs = _prep(inputs)

    percore = {}
    xh = x.astype(np.float16).reshape(NC_, BL, NK, PF, HW).transpose(0, 2, 3, 1, 4)
    percore["x16"] = np.ascontiguousarray(xh.reshape(NC_ * NK, PF, BL, HW))
    for name, a in arrs.items():
        percore[name] = np.concatenate([a] * NC_, axis=0)

    args = [jax.device_put(percore[n], sharding) for n in in_names]
    zeros = [jax.device_put(np.zeros((NC_ * s[0], *s[1:]), d), sharding)
             for (s, d) in out_shapes]
    outs = fn(*args, *zeros)
    y16 = np.asarray(outs[out_names.index("y16")])            # [8*NK, PF, BL, HW]
    y16 = y16.reshape(NC_, NK, PF, BL, HW).transpose(0, 3, 1, 2, 4)
    y = y16.reshape(B, C, H, W).astype(np.float32)
    y += x
    return y

